# revision 1
# baseline (speedup 1.0000x reference)
"""Trainium2 Bass kernel for the 2-layer "bidirectional" (dual forward) GRU encoder.

Contract: kernel(**inputs) takes FULL unsharded inputs (numpy/jax arrays) and
returns the FULL output [1, B, 2H].  Internally shards the batch across 8
NeuronCores (data-parallel, weights replicated), runs a Bass/Tile program per
core via run_bass_kernel_spmd, and gathers on host.

Per core (Bs = B/8 = 8 batch rows), phases:
  1. gi0 = W_ih0 @ x + biases          (big matmuls -> gi0 DRAM buffer)
  2. layer-0 recurrence over T steps   (single For_i, 32-step body)
  3. gi1 = W_ih1 @ out0 + biases       (big matmuls -> gi1 DRAM buffer)
  4. layer-1 recurrence                (history -> out1 output)
Host picks h at t = seq_len[b]-1 per batch row from out1.

Recurrence step (gate-major: partitions = gate dim, free = batch):
  gh = W_hh @ h[t-1]   24 128x128x8 matmuls into PSUM [128, 12, 8]
  rz = sigmoid(gi_rz + gh_rz)
  n  = tanh(gi_n + r * (gh_n + b_hn))
  h[t] = n + z * (h[t-1] - n)

Gate-chunk index g = m*2 + d (m in 0..5 = r0,r1,z0,z1,n0,n1; d = direction),
so r/z/n slices are contiguous.  h layout: col (j*2+d)*8 + b (j = H-chunk).

Dynamic (register-offset) APs are confined to 3 DMAs per loop body (gi stage-in
x2, history flush) — engine-side dynamic APs exhaust engine registers and the
walrus here rejects them in quantity.
"""

import copy

import numpy as np

import concourse.bass as bass
import concourse.tile as tile
from concourse import mybir
from concourse.bass import ds
from concourse.bass_utils import run_bass_kernel_spmd

# problem constants (hardcoded per harness contract)
B, T, I, H = 64, 1024, 64, 256
NCORES = 8
BS = B // NCORES          # 8 batch rows per core
G = 3 * H                 # 768 gates per direction
UNROLL = 32               # steps per For_i iteration
STAGE = 16                # steps per gi stage half
GRP = 512                 # tokens per gi-production matmul group
FP = mybir.dt.float32
AF = mybir.ActivationFunctionType
PAD = 2 * UNROLL          # stage lookahead past T (reads garbage, never used)


MAXW = 1  # max sync-waits walrus accepts per instruction here


def _nop_template():
    scratch = bass.Bass()
    return scratch.vector.nop().ins


def split_multiwait_insts(nc):
    """Walrus in this container rejects more than MAXW sync-waits on an
    instruction ("Too many sync wait commands").  Move excess waits onto
    preceding same-engine NoOp carriers."""
    tmpl = _nop_template()
    n_split = 0
    uid = [0]
    for f in nc.m.functions:
        for bb in f.blocks:
            out = []
            for inst in bb.instructions:
                si = getattr(inst, "sync_info", None)
                if si is not None and si.on_wait and len(si.on_wait) > MAXW:
                    waits = list(si.on_wait)
                    excess, keep = waits[:-MAXW], waits[-MAXW:]
                    for i in range(0, len(excess), MAXW):
                        cl = copy.copy(tmpl)
                        cl.name = f"nopw-{uid[0]}"
                        uid[0] += 1
                        cl.engine = inst.engine
                        cl.sync_info = mybir.SyncInfo(
                            on_wait=excess[i : i + MAXW], on_update=[]
                        )
                        out.append(cl)
                    si.on_wait = keep
                    n_split += 1
                out.append(inst)
            bb.instructions[:] = out
    return n_split


def build_program(n_steps=T):
    """Build the SPMD Bass program (shared by all 8 cores)."""
    TT = n_steps
    NTOK = TT * BS
    NPTOK = (TT + PAD) * BS  # padded for stage lookahead
    nc = bass.Bass()

    x_d = nc.declare_dram_parameter("x", [I, NTOK], FP, isOutput=False)
    whh_d = [
        nc.declare_dram_parameter(f"whh{l}", [128, 2 * 2 * 6 * 128], FP, isOutput=False)
        for l in range(2)
    ]
    wih0_d = nc.declare_dram_parameter("wih0T", [I, 2 * 6 * 128], FP, isOutput=False)
    wih1_d = nc.declare_dram_parameter("wih1T", [128, 2 * 4 * 6 * 128], FP, isOutput=False)
    bgi_d = [
        nc.declare_dram_parameter(f"bias_gi{l}", [128, 12], FP, isOutput=False)
        for l in range(2)
    ]
    bhn_d = [
        nc.declare_dram_parameter(f"bias_hn{l}", [128, 4 * BS], FP, isOutput=False)
        for l in range(2)
    ]
    gsel_d = nc.declare_dram_parameter("gsel", [BS, 1], mybir.dt.int32, isOutput=False)
    hidden_d = nc.declare_dram_parameter("hidden", [BS, 512], FP, isOutput=True)

    with tile.TileContext(nc) as tc:
        with (
            tc.tile_pool(name="const", bufs=1) as const,
            tc.tile_pool(name="dram", bufs=1, space=bass.MemorySpace.DRAM) as dram,
            tc.tile_pool(name="go", bufs=3) as go_pool,
            tc.tile_pool(name="xin", bufs=3) as x_pool,
            tc.tile_pool(name="ring", bufs=2) as ring_pool,
            tc.tile_pool(name="rhs1", bufs=3) as rhs1_pool,
            tc.tile_pool(name="work", bufs=3) as work,
            tc.tile_pool(name="gipsum", bufs=2, space=bass.MemorySpace.PSUM) as gi_psum,
            tc.tile_pool(name="stpsum", bufs=2, space=bass.MemorySpace.PSUM) as st_psum,
        ):
            # ---- constants ----
            whh_sb = []
            for l in range(2):
                t_ = const.tile([128, 2, 2, 6, 128], FP, tag=f"whh{l}")
                nc.sync.dma_start(
                    t_[:], whh_d[l].rearrange("p (d k m q) -> p d k m q", d=2, k=2, m=6)
                )
                whh_sb.append(t_)
            wih0_sb = const.tile([I, 2, 6, 128], FP, tag="wih0")
            nc.sync.dma_start(wih0_sb[:], wih0_d.rearrange("p (d m q) -> p d m q", d=2, m=6))
            wih1_sb = const.tile([128, 2, 4, 6, 128], FP, tag="wih1")
            nc.sync.dma_start(wih1_sb[:], wih1_d.rearrange("p (d k m q) -> p d k m q", d=2, k=4, m=6))
            bgi_sb, bhn_sb = [], []
            for l in range(2):
                bg = const.tile([128, 12], FP, tag=f"bgi{l}")
                nc.sync.dma_start(bg[:], bgi_d[l][:])
                bgi_sb.append(bg)
                bh = const.tile([128, 4, BS], FP, tag=f"bhn{l}")
                nc.sync.dma_start(bh[:], bhn_d[l].rearrange("p (c b) -> p c b", c=4))
                bhn_sb.append(bh)
            h_carry = const.tile([128, 4 * BS], FP, tag="hcarry")
            gstage = [
                const.tile([128, 12, 2, STAGE * BS], FP, tag=f"gstage{l}", name=f"gstage{l}")
                for l in range(2)
            ]

            out0_d = dram.tile([4, 128, NTOK], FP)
            hist2_d = dram.tile([NTOK, 512], FP)  # layer-1 history, token-major
            gi_d = [dram.tile([12, 128, NPTOK], FP, tag=f"gi{l}d", name=f"gi{l}d") for l in range(2)]

            n_grp = NTOK // GRP

            # ================= phase 1: gi0 =================
            for grp in range(n_grp):
                t0 = grp * GRP
                x_sb = x_pool.tile([I, GRP], FP, tag="x")
                nc.sync.dma_start(x_sb[:], x_d[:, t0 : t0 + GRP])
                go = go_pool.tile([128, 12, GRP], FP, tag="go")
                for m in range(6):
                    for d in range(2):
                        g = m * 2 + d
                        ps = gi_psum.tile([128, GRP], FP, tag="gips")
                        nc.tensor.matmul(
                            ps[:], wih0_sb[:, d, m, :], x_sb[:],
                            start=True, stop=True,
                        )
                        nc.scalar.activation(
                            go[:, g, :], ps[:], AF.Identity,
                            bias=bgi_sb[0][:, g : g + 1],
                        )
                nc.sync.dma_start(gi_d[0][:, :, t0 : t0 + GRP].rearrange("g p t -> p g t"), go[:])

            # ================= recurrence =================
            def recurrence(layer, hist_view):
                whh = whh_sb[layer]
                bhn = bhn_sb[layer]
                gst = gstage[layer]
                gid = gi_d[layer]
                nc.vector.memset(h_carry[:], 0.0)
                # prologue: stage steps 0..2*STAGE-1
                for par in range(2):
                    c0 = par * STAGE * BS
                    nc.sync.dma_start(
                        gst[:, :, par, :],
                        gid[:, :, c0 : c0 + STAGE * BS].rearrange("g p t -> p g t"),
                    )
                with tc.For_i(0, TT, UNROLL) as iv:
                    ring = ring_pool.tile([128, 4, UNROLL, BS], FP, tag=f"ring{layer}")
                    for u in range(UNROLL):
                        par = (u // STAGE) % 2
                        o8 = (u % STAGE) * BS
                        h_prev = h_carry[:, :] if u == 0 else ring[:, :, u - 1, :]
                        ps = st_psum.tile([128, 12, BS], FP, tag=f"stps{layer}")
                        # rz gates (m 0..3) first so their elementwise tail
                        # overlaps the n-gate matmuls; 4x col-tiled so the 4
                        # 32-col weight loads stream concurrently.
                        for m in range(6):
                            for d in range(2):
                                g = m * 2 + d
                                for k in range(2):
                                    hk = (k * 2 + d) * BS
                                    rhs = (
                                        h_carry[:, hk : hk + BS] if u == 0
                                        else ring[:, k * 2 + d, u - 1, :]
                                    )
                                    nc.tensor.matmul(
                                        ps[:, g, :],
                                        whh[:, d, k, m, :],
                                        rhs,
                                        start=(k == 0), stop=(k == 1),
                                    )
                        arz = work.tile([128, 8, BS], FP, tag="arz")
                        nc.vector.tensor_add(
                            arz[:], gst[:, 0:8, par, o8 : o8 + BS], ps[:, 0:8, :]
                        )
                        rzs = work.tile([128, 8, BS], FP, tag="rzs")
                        nc.scalar.activation(rzs[:], arz[:], AF.Sigmoid)
                        ghn = work.tile([128, 4, BS], FP, tag="ghn")
                        nc.vector.tensor_add(ghn[:], ps[:, 8:12, :], bhn[:])
                        t1 = work.tile([128, 4, BS], FP, tag="t1")
                        nc.vector.tensor_mul(t1[:], rzs[:, 0:4, :], ghn[:])
                        an = work.tile([128, 4, BS], FP, tag="an")
                        nc.vector.tensor_add(
                            an[:], gst[:, 8:12, par, o8 : o8 + BS], t1[:]
                        )
                        nt = work.tile([128, 4, BS], FP, tag="nt")
                        nc.scalar.activation(nt[:], an[:], AF.Tanh)
                        # h' = n*(1-z) + z*h; zq/q2 run during the tanh window
                        zq = work.tile([128, 4, BS], FP, tag="zq")
                        nc.scalar.activation(
                            zq[:], rzs[:, 4:8, :], AF.Identity, bias=1.0, scale=-1.0
                        )
                        q2 = work.tile([128, 4, BS], FP, tag="q2")
                        nc.vector.tensor_mul(q2[:], rzs[:, 4:8, :], h_prev)
                        q3 = work.tile([128, 4, BS], FP, tag="q3")
                        nc.vector.tensor_mul(q3[:], nt[:], zq[:])
                        nc.vector.tensor_add(ring[:, :, u, :], q3[:], q2[:])

                        if u == STAGE - 1:
                            # refill par0 with steps iv+UNROLL .. +UNROLL+STAGE-1
                            nc.sync.dma_start(
                                gst[:, :, 0, :],
                                gid[:, :, ds((iv + UNROLL) * BS, STAGE * BS)].rearrange(
                                    "g p t -> p g t"
                                ),
                            )
                    # end of body
                    nc.vector.tensor_copy(h_carry[:], ring[:, :, UNROLL - 1, :])
                    nc.sync.dma_start(
                        gst[:, :, 1, :],
                        gid[:, :, ds((iv + UNROLL + STAGE) * BS, STAGE * BS)].rearrange(
                            "g p t -> p g t"
                        ),
                    )
                    # flush history
                    for d in range(2):
                        for j in range(2):
                            co = (j * 2 + d) * BS
                            if layer == 0:
                                # -> out0[d*2+j, p, tok] (feature-major for gi1)
                                nc.sync.dma_start(
                                    hist_view[d * 2 + j, :, ds(iv * BS, UNROLL * BS)],
                                    ring[:, j * 2 + d, :, :],
                                )
                            else:
                                # -> hist2[tok, (d*2+j)*128 + p] (token-major
                                # so the final gather is a row gather)
                                hv = hist_view.rearrange(
                                    "(s b) (c p) -> c p s b", b=BS, p=128
                                )
                                nc.sync.dma_start(
                                    hv[d * 2 + j, :, ds(iv, UNROLL), :],
                                    ring[:, j * 2 + d, :, :],
                                )

            recurrence(0, out0_d)

            # ================= phase 3: gi1 =================
            for grp in range(n_grp):
                t0 = grp * GRP
                rhs = rhs1_pool.tile([128, 4, GRP], FP, tag="rhs1")
                for k in range(4):
                    nc.sync.dma_start(rhs[:, k, :], out0_d[k, :, t0 : t0 + GRP])
                go = go_pool.tile([128, 12, GRP], FP, tag="go")
                for m in range(6):
                    for d in range(2):
                        g = m * 2 + d
                        ps = gi_psum.tile([128, GRP], FP, tag="gips")
                        for k in range(4):
                            nc.tensor.matmul(
                                ps[:], wih1_sb[:, d, k, m, :], rhs[:, k, :],
                                start=(k == 0), stop=(k == 3),
                            )
                        nc.scalar.activation(
                            go[:, g, :], ps[:], AF.Identity,
                            bias=bgi_sb[1][:, g : g + 1],
                        )
                nc.sync.dma_start(gi_d[1][:, :, t0 : t0 + GRP].rearrange("g p t -> p g t"), go[:])

            recurrence(1, hist2_d)

            # ---- final gather: hidden[b, :] = hist2[gsel[b], :] ----
            sel_sb = const.tile([BS, 1], mybir.dt.int32, tag="sel")
            nc.sync.dma_start(sel_sb[:], gsel_d[:])
            gath = const.tile([BS, 512], FP, tag="gath")
            nc.gpsimd.indirect_dma_start(
                out=gath[:],
                out_offset=None,
                in_=hist2_d[:],
                in_offset=bass.IndirectOffsetOnAxis(ap=sel_sb[:, :1], axis=0),
            )
            nc.sync.dma_start(hidden_d[:], gath[:])

    split_multiwait_insts(nc)
    return nc


def prep_host_inputs(input_tensor, W_ih0, W_hh0, b_ih0, b_hh0,
                     W_ih1, W_hh1, b_ih1, b_hh1, n_steps=T, seq_len=None):
    """Build the per-core in_maps (weights replicated, x sharded)."""
    x = np.asarray(input_tensor, np.float32)
    if seq_len is None:
        idx = np.zeros(B, np.int64)
    else:
        idx = np.clip(np.asarray(seq_len).astype(np.int64) - 1, 0, n_steps - 1)
    W_ih0 = np.asarray(W_ih0, np.float32); W_hh0 = np.asarray(W_hh0, np.float32)
    b_ih0 = np.asarray(b_ih0, np.float32); b_hh0 = np.asarray(b_hh0, np.float32)
    W_ih1 = np.asarray(W_ih1, np.float32); W_hh1 = np.asarray(W_hh1, np.float32)
    b_ih1 = np.asarray(b_ih1, np.float32); b_hh1 = np.asarray(b_hh1, np.float32)

    def whh_tiles(W):  # [2, 768, 256] -> [128, (d k m q)]
        a = W.reshape(2, 6, 128, 2, 128)          # d, m, q, k, p
        a = a.transpose(4, 0, 3, 1, 2)            # p, d, k, m, q
        return np.ascontiguousarray(a.reshape(128, -1))

    def wih_tiles(W, kin):
        inn = W.shape[2]
        if kin == 0:  # layer 0: K = I = 64, single chunk
            a = W.reshape(2, 6, 128, inn)         # d, m, q, p
            a = a.transpose(3, 0, 1, 2)           # p, d, m, q
            return np.ascontiguousarray(a.reshape(inn, -1))
        a = W.reshape(2, 6, 128, kin, 128)        # d, m, q, k, p
        a = a.transpose(4, 0, 3, 1, 2)            # p, d, k, m, q
        return np.ascontiguousarray(a.reshape(128, -1))

    def bias_gi(b_ih, b_hh):  # [128, 12], g = m*2+d; rz rows get b_ih+b_hh
        out = np.zeros((128, 12), np.float32)
        for m in range(6):
            for d in range(2):
                seg = slice(m * 128, (m + 1) * 128)
                out[:, m * 2 + d] = b_ih[d, seg] + (b_hh[d, seg] if m < 4 else 0.0)
        return out

    def bias_hn(b_hh):  # [128, (j*2+d)*BS+b] replicated over batch
        out = np.zeros((128, 4, BS), np.float32)
        for j in range(2):
            for d in range(2):
                out[:, j * 2 + d, :] = b_hh[d, 512 + j * 128 : 512 + (j + 1) * 128, None]
        return out.reshape(128, -1)

    shared = {
        "whh0": whh_tiles(W_hh0), "whh1": whh_tiles(W_hh1),
        "wih0T": wih_tiles(W_ih0, 0), "wih1T": wih_tiles(W_ih1, 4),
        "bias_gi0": bias_gi(b_ih0, b_hh0), "bias_gi1": bias_gi(b_ih1, b_hh1),
        "bias_hn0": bias_hn(b_hh0), "bias_hn1": bias_hn(b_hh1),
    }
    in_maps = []
    for c in range(NCORES):
        xs = x[c * BS : (c + 1) * BS, :n_steps, :]        # [BS, TT, I]
        xs = np.ascontiguousarray(xs.transpose(2, 1, 0).reshape(I, n_steps * BS))
        gsel = (idx[c * BS : (c + 1) * BS] * BS + np.arange(BS)).astype(np.int32)
        in_maps.append({"x": xs, "gsel": gsel.reshape(BS, 1), **shared})
    return in_maps


_cached = {}


def run_spmd(in_maps, n_steps=T, **kwargs):
    if n_steps not in _cached:
        _cached[n_steps] = build_program(n_steps)
    return run_bass_kernel_spmd(
        _cached[n_steps], in_maps, core_ids=list(range(NCORES)), **kwargs
    )


def kernel(input_tensor, W_ih0, W_hh0, b_ih0, b_hh0,
           W_ih1, W_hh1, b_ih1, b_hh1, seq_len):
    in_maps = prep_host_inputs(input_tensor, W_ih0, W_hh0, b_ih0, b_hh0,
                               W_ih1, W_hh1, b_ih1, b_hh1, seq_len=seq_len)
    res = run_spmd(in_maps)
    out = np.zeros((1, B, 2 * H), np.float32)
    for c in range(NCORES):
        out[0, c * BS : (c + 1) * BS, :] = res.results[c]["hidden"]
    return out

